# revision 19
# baseline (speedup 1.0000x reference)
"""LSTM-cell scan kernel for Trainium2 (8 NeuronCores, data-parallel over batch).

Problem: T=512 sequential LSTMCell steps, B=4096, I=10, H=20 (gates G=80).
Sharding: batch 4096 -> 8 cores x 512. Weights replicated. No cross-core comm.

v2 design: two staggered half-batch chains per core (2 x 256 batch) so the
irreducible per-step recurrence latency of one chain hides under the other's
engine work.  All of x lives in SBUF (loaded once, bf16), embedded in a giant
per-half "hb" tensor [128, T+1, 64] whose sub-slot t holds [x_aug(t) | h(t-1)]
per 32-col block; h is written in place each step, so there are ZERO per-step
DMAs.  Output h is DMA'd out once at the end (bf16; host upcasts).

Per half-chain step (blocks b=0,1 of 128 batch on partitions):
  PE:   transpose hb[:,t,:] -> pT [64,128]; 2 row-grouped matmuls
        (stationary = slot rows 32b, moving = replicated W_aug^T) -> psum gates
  Act:  Sigmoid over [128,2,80] (g-rows pre-scaled by 2: tanh(g)=2*sig(2g)-1),
        Tanh over c [128,2,20]
  DVE:  W=(Sg-.5)*Si ; C=2W+FC ; h=So*TH into hb h-cols ; slot copy psum->sbuf
  Pool: FC=Sf*C
"""

import os
import sys

import numpy as np

sys.path.insert(0, "/opt/trn_rl_repo")

T, BFULL, I, H = 512, 4096, 10, 20
NCORES = 8
B = BFULL // NCORES  # 512 per core
G = 4 * H  # 80
IA = I + 2  # x augmented with ones column (bias row) + zero pad col
KAUG = IA + H  # 32
NHALF = 2  # chains per core
NBLK = 2  # batch blocks of 128 per chain

_compiled = None

_MAXW = 1  # max sem waits this walrus accepts attached to one instruction


def _split_waits(nc):
    """Hoist attached sem waits into standalone EventSemaphore instructions.

    This walrus build rejects instructions carrying more than one sync wait
    ("Too many sync wait commands").  For any instruction with multiple
    waits, emit standalone single-wait EventSemaphore instructions directly
    before it in the same engine stream (equivalent semantics: the engine
    queue stalls on each in turn).
    """
    from concourse import mybir

    for bbb in nc.bb_map.values():
        inner = bbb.bb
        insts = list(inner.instructions)
        out = []
        changed = False
        for inst in insts:
            si = getattr(inst, "sync_info", None)
            if si is not None and si.on_wait and len(si.on_wait) > _MAXW:
                waits = list(si.on_wait)
                for w in waits[:-_MAXW]:
                    ev = mybir.InstEventSemaphore(
                        name=nc.get_next_instruction_name(),
                        ins=[],
                        outs=[],
                    )
                    ev.engine = inst.engine
                    ev.sync_info = mybir.SyncInfo(on_wait=[w], on_update=[])
                    nc.register_instruction(ev)
                    out.append(ev)
                inst.sync_info = mybir.SyncInfo(
                    on_wait=waits[-_MAXW:], on_update=list(si.on_update or [])
                )
                changed = True
            out.append(inst)
        if changed:
            inner.instructions = out


def _build_bass(nsteps=T, variant=""):
    import concourse.bass as bass
    import concourse.tile as tile
    from concourse import mybir
    from concourse.masks import make_identity

    f32 = mybir.dt.float32
    bf16 = mybir.dt.bfloat16
    AF = mybir.ActivationFunctionType
    OP = mybir.AluOpType

    T_ = nsteps
    nc = bass.Bass()

    # DRAM tensors, all pre-packed on host:
    #  x{m}:  [128, T+1, 64] bf16 — the FULL hb image, partition-major:
    #         sub-slot t cols 32b+{0:12} = x_aug(t), sub-slot 0 cols
    #         32b+{12:32} = hx, everything else 0.  One fat contiguous DMA.
    #  cx{m}: [128, 2, 20] f32
    #  wT4:   [128, G] bf16  (W_aug^T replicated at 32-row offsets, g rows x2)
    #  hs{m}: [128, T, 64] bf16 out — raw dump of sub-slots 1..T (h at cols
    #         32b+{12:32}; host slices).
    x_d = [
        nc.dram_tensor(
            f"x{m}", [128, T_ + 1, NBLK * KAUG], bf16, kind="ExternalInput"
        )
        for m in range(NHALF)
    ]
    cx_d = [
        nc.dram_tensor(f"cx{m}", [128, NBLK, H], f32, kind="ExternalInput")
        for m in range(NHALF)
    ]
    w_d = nc.dram_tensor("wT4", [128, G], bf16, kind="ExternalInput")
    hs_d = [
        nc.dram_tensor(
            f"hs{m}", [128, T_, NBLK * KAUG], bf16, kind="ExternalOutput"
        )
        for m in range(NHALF)
    ]

    fc_engine = "vector" if "fcdve" in variant else "gpsimd"
    copy_engine = "gpsimd" if "cpgps" in variant else "vector"

    with tile.TileContext(nc) as tc:
        with (
            tc.tile_pool(name="const", bufs=1) as const,
            tc.tile_pool(name="slotA", bufs=2) as slotpA,
            tc.tile_pool(name="slotB", bufs=2) as slotpB,
            tc.tile_pool(name="sgA", bufs=2) as sgpA,
            tc.tile_pool(name="sgB", bufs=2) as sgpB,
            tc.tile_pool(name="fcA", bufs=2) as fcpA,
            tc.tile_pool(name="fcB", bufs=2) as fcpB,
            tc.tile_pool(name="wpA", bufs=2) as wpA,
            tc.tile_pool(name="wpB", bufs=2) as wpB,
            tc.tile_pool(name="thA", bufs=2) as thpA,
            tc.tile_pool(name="thB", bufs=2) as thpB,
            tc.tile_pool(name="psgA", bufs=1, space="PSUM") as psgA,
            tc.tile_pool(name="psgB", bufs=1, space="PSUM") as psgB,
            tc.tile_pool(name="pstA", bufs=1, space="PSUM") as pstA,
            tc.tile_pool(name="pstB", bufs=1, space="PSUM") as pstB,
        ):
            slotp = [slotpA, slotpB]
            sgp = [sgpA, sgpB]
            fcp = [fcpA, fcpB]
            wp = [wpA, wpB]
            thp = [thpA, thpB]
            psg = [psgA, psgB]
            pst = [pstA, pstB]

            # ---- constants ----
            ident = const.tile([128, 128], bf16)
            make_identity(nc, ident)
            wb = const.tile([128, G], bf16)
            nc.sync.dma_start(out=wb, in_=w_d[:, :])

            # persistent cell state per half [128, 2, 20] f32
            C = []
            for m in range(NHALF):
                cm = const.tile([128, NBLK, H], f32, name=f"C{m}")
                nc.sync.dma_start(out=cm, in_=cx_d[m][:, :, :])
                C.append(cm)

            # giant hb per half: [128, T+1, 64] bf16.
            # sub-slot t cols (b,q): q 0:12 = x_aug(t), q 12:32 = h(t-1).
            HB = []
            for m in range(NHALF):
                hbm = const.tile([128, T_ + 1, NBLK * KAUG], bf16, name=f"HB{m}")
                HB.append(hbm)

            def hb_slot(m, t):
                return HB[m][:, t, :]  # [128, 64]



            def hb_h(m, t):
                # h-cols of sub-slot t: [128, 2, 20]
                return HB[m].rearrange("p t (b q) -> p t b q", b=NBLK)[
                    :, t, :, IA:KAUG
                ]

            # ---- prologue: load the full hb image (x + hx + zeros) ----
            # chunked along T: a single DMA's element-count field is 16-bit
            xchunk = 128
            for m in range(NHALF):
                for t0 in range(0, T_ + 1, xchunk):
                    t1 = min(t0 + xchunk, T_ + 1)
                    nc.sync.dma_start(
                        out=HB[m][:, t0:t1, :], in_=x_d[m][:, t0:t1, :]
                    )

            # half m's transposed slot lives at partitions 64m:64m+64 so the
            # gates matmul's lhsT partition start matches its wb slice
            # (walrus: "Fmap and Weight must start at the same partition").
            def make_slot(m, t):
                pT = pst[m].tile([128, 128], bf16, tag=f"pst{m}")
                nc.tensor.transpose(
                    pT[64 * m : 64 * m + 64, :],
                    hb_slot(m, t),
                    ident,
                    tile_position=(0, 64 * m),
                )
                sl = slotp[m].tile([128, 128], bf16, tag=f"slot{m}")
                getattr(nc, copy_engine).tensor_copy(
                    sl[64 * m : 64 * m + 64, :], pT[64 * m : 64 * m + 64, :]
                )
                return sl

            slot = [make_slot(m, 0) for m in range(NHALF)]

            # ---- main loop, fully unrolled, halves interleaved ----
            for t in range(T_):
                for m in range(NHALF):
                    # gates matmuls: 2 row-grouped blocks -> 2 psum banks
                    pg = psg[m].tile([128, NBLK, 512], f32, tag=f"psg{m}")
                    for b in range(NBLK):
                        r = 64 * m + 32 * b
                        nc.tensor.matmul(
                            pg[:, b, 0:G],
                            lhsT=slot[m][r : r + KAUG, :],
                            rhs=wb[r : r + KAUG, :],
                            start=True,
                            stop=True,
                            tile_position=(r, 0),
                        )
                    # sigmoid over all gates (g rows pre-scaled by 2)
                    S = sgp[m].tile([128, NBLK, G], bf16, tag=f"sg{m}")
                    nc.scalar.activation(S, pg[:, :, 0:G], AF.Sigmoid)
                    Si = S[:, :, 0:20]
                    Sf = S[:, :, 20:40]
                    Sg = S[:, :, 40:60]
                    So = S[:, :, 60:80]

                    # c update: fc = f*c ; w = (sg-0.5)*i ; c = 2w + fc
                    FC = fcp[m].tile([128, NBLK, H], f32, tag=f"fc{m}")
                    getattr(nc, fc_engine).tensor_mul(FC, Sf, C[m])
                    W = wp[m].tile([128, NBLK, H], bf16, tag=f"wp{m}")
                    nc.vector.scalar_tensor_tensor(
                        W, in0=Sg, scalar=0.5, in1=Si, op0=OP.subtract, op1=OP.mult
                    )
                    nc.vector.scalar_tensor_tensor(
                        C[m], in0=W, scalar=2.0, in1=FC, op0=OP.mult, op1=OP.add
                    )
                    TH = thp[m].tile([128, NBLK, H], bf16, tag=f"th{m}")
                    nc.scalar.activation(TH, C[m], AF.Tanh)

                    # h(t) = o * tanh(c) -> hb sub-slot t+1 h-cols (also = hs[t])
                    nc.vector.tensor_mul(hb_h(m, t + 1), So, TH)

                    if t + 1 < T_:
                        slot[m] = make_slot(m, t + 1)

            # ---- epilogue: raw dump of sub-slots 1..T, chunked ----
            for m in range(NHALF):
                for t0 in range(0, T_, xchunk):
                    t1 = min(t0 + xchunk, T_)
                    nc.sync.dma_start(
                        out=hs_d[m][:, t0:t1, :],
                        in_=HB[m][:, t0 + 1 : t1 + 1, :],
                    )

    _split_waits(nc)
    return nc


def _get_compiled():
    global _compiled
    if _compiled is None:
        _compiled = _build_bass()
    return _compiled


def _prep_w(W_ih, W_hh, b_ih, b_hh):
    import ml_dtypes

    # augmented weight [G, KAUG]: cols 0:10 = W_ih, col 10 = bias,
    # col 11 = zero pad, cols 12:32 = W_hh
    Waug = np.zeros((G, KAUG), dtype=np.float32)
    Waug[:, 0:I] = W_ih
    Waug[:, I] = b_ih + b_hh
    Waug[:, IA:] = W_hh
    Waug[40:60, :] *= 2.0  # g rows: sigmoid(2g) trick
    wT4 = np.zeros((128, G), dtype=np.float32)
    for a in range(4):
        wT4[32 * a : 32 * a + KAUG, :] = Waug.T
    return wT4.astype(ml_dtypes.bfloat16)


def build_in_maps(x, hx, cx, W_ih, W_hh, b_ih, b_hh):
    """Host-side packing: per-core, per-half tensors as the kernel expects."""
    import ml_dtypes

    bf16 = ml_dtypes.bfloat16
    x = np.asarray(x, np.float32)
    hx = np.asarray(hx, np.float32)
    cx = np.asarray(cx, np.float32)
    wT4 = _prep_w(
        np.asarray(W_ih, np.float32),
        np.asarray(W_hh, np.float32),
        np.asarray(b_ih, np.float32),
        np.asarray(b_hh, np.float32),
    )

    # Full hb image: [core, half, 128(p), T+1, 2(b), 32]
    # sub-slot t cols 32b+{0:12} = x_aug(t) (t<T); sub-slot 0 cols
    # 32b+{12:32} = hx; else 0.  batch = core*512 + half*256 + block*128 + p
    img = np.zeros((NCORES, NHALF, 128, T + 1, NBLK, KAUG), dtype=np.float32)
    # x: [T, 4096, 10] -> [core, half, p, t, b, i]
    x6 = x.reshape(T, NCORES, NHALF, NBLK, 128, I).transpose(1, 2, 4, 0, 3, 5)
    img[:, :, :, :T, :, 0:I] = x6
    img[:, :, :, :T, :, I] = 1.0
    h6 = hx.reshape(NCORES, NHALF, NBLK, 128, H).transpose(0, 1, 3, 2, 4)
    img[:, :, :, 0, :, IA:KAUG] = h6
    img = np.ascontiguousarray(img).astype(bf16)

    c6 = cx.reshape(NCORES, NHALF, NBLK, 128, H).transpose(0, 1, 3, 2, 4)
    c6 = np.ascontiguousarray(c6).astype(np.float32)

    in_maps = []
    for k in range(NCORES):
        im = {"wT4": wT4}
        for m in range(NHALF):
            im[f"x{m}"] = img[k, m].reshape(128, T + 1, NBLK * KAUG)
            im[f"cx{m}"] = c6[k, m]
        in_maps.append(im)
    return in_maps


def unshard_output(results):
    """results: list per core of {hs0, hs1: [128,T,64] bf16} -> [T,4096,20] f32."""
    outs = np.stack(
        [
            np.stack([np.asarray(results[k][f"hs{m}"]) for m in range(NHALF)])
            for k in range(NCORES)
        ]
    ).reshape(NCORES, NHALF, 128, T, NBLK, KAUG)[:, :, :, :, :, IA:KAUG]
    # [core, half, 128(p), T, 2(b), 20]
    outs = outs.astype(np.float32).transpose(3, 0, 1, 4, 2, 5)
    # -> [T, core, half, 2(b), 128(p), 20]
    return np.ascontiguousarray(outs.reshape(T, BFULL, H))


def kernel(x, hx, cx, W_ih, W_hh, b_ih, b_hh):
    from concourse.bass_utils import run_bass_kernel_spmd

    nc = _get_compiled()
    in_maps = build_in_maps(x, hx, cx, W_ih, W_hh, b_ih, b_hh)
    res = run_bass_kernel_spmd(nc, in_maps, list(range(NCORES)))
    return unshard_output(res.results)


# revision 23
# speedup vs baseline: 1.0241x; 1.0241x over previous
"""LSTM-cell scan kernel for Trainium2 (8 NeuronCores, data-parallel over batch).

Problem: T=512 sequential LSTMCell steps, B=4096, I=10, H=20 (gates G=80).
Sharding: batch 4096 -> 8 cores x 512. Weights replicated. No cross-core comm.

v2 design: two staggered half-batch chains per core (2 x 256 batch) so the
irreducible per-step recurrence latency of one chain hides under the other's
engine work.  All of x lives in SBUF (loaded once, bf16), embedded in a giant
per-half "hb" tensor [128, T+1, 64] whose sub-slot t holds [x_aug(t) | h(t-1)]
per 32-col block; h is written in place each step, so there are ZERO per-step
DMAs.  Output h is DMA'd out once at the end (bf16; host upcasts).

Per half-chain step (blocks b=0,1 of 128 batch on partitions):
  PE:   transpose hb[:,t,:] -> pT [64,128]; 2 row-grouped matmuls
        (stationary = slot rows 32b, moving = replicated W_aug^T) -> psum gates
  Act:  Sigmoid over [128,2,80] (g-rows pre-scaled by 2: tanh(g)=2*sig(2g)-1),
        Tanh over c [128,2,20]
  DVE:  W=(Sg-.5)*Si ; C=2W+FC ; h=So*TH into hb h-cols ; slot copy psum->sbuf
  Pool: FC=Sf*C
"""

import os
import sys

import numpy as np

sys.path.insert(0, "/opt/trn_rl_repo")

T, BFULL, I, H = 512, 4096, 10, 20
NCORES = 8
B = BFULL // NCORES  # 512 per core
G = 4 * H  # 80
IA = I + 2  # x augmented with ones column (bias row) + zero pad col
KAUG = IA + H  # 32
NHALF = 2  # chains per core
NBLK = 2  # batch blocks of 128 per chain

_compiled = None

_MAXW = 1  # max sem waits this walrus accepts attached to one instruction


def _split_waits(nc):
    """Hoist attached sem waits into standalone EventSemaphore instructions.

    This walrus build rejects instructions carrying more than one sync wait
    ("Too many sync wait commands").  For any instruction with multiple
    waits, emit standalone single-wait EventSemaphore instructions directly
    before it in the same engine stream (equivalent semantics: the engine
    queue stalls on each in turn).
    """
    from concourse import mybir

    for bbb in nc.bb_map.values():
        inner = bbb.bb
        insts = list(inner.instructions)
        out = []
        changed = False
        for inst in insts:
            si = getattr(inst, "sync_info", None)
            if si is not None and si.on_wait and len(si.on_wait) > _MAXW:
                waits = list(si.on_wait)
                for w in waits[:-_MAXW]:
                    ev = mybir.InstEventSemaphore(
                        name=nc.get_next_instruction_name(),
                        ins=[],
                        outs=[],
                    )
                    ev.engine = inst.engine
                    ev.sync_info = mybir.SyncInfo(on_wait=[w], on_update=[])
                    nc.register_instruction(ev)
                    out.append(ev)
                inst.sync_info = mybir.SyncInfo(
                    on_wait=waits[-_MAXW:], on_update=list(si.on_update or [])
                )
                changed = True
            out.append(inst)
        if changed:
            inner.instructions = out


def _build_bass(nsteps=T, variant=""):
    import concourse.bass as bass
    import concourse.tile as tile
    from concourse import mybir
    from concourse.masks import make_identity

    f32 = mybir.dt.float32
    bf16 = mybir.dt.bfloat16
    AF = mybir.ActivationFunctionType
    OP = mybir.AluOpType

    T_ = nsteps
    nc = bass.Bass()

    # DRAM tensors, all pre-packed on host:
    #  x{m}:  [128, T+1, 64] bf16 — the FULL hb image, partition-major:
    #         sub-slot t cols 32b+{0:12} = x_aug(t), sub-slot 0 cols
    #         32b+{12:32} = hx, everything else 0.  One fat contiguous DMA.
    #  cx{m}: [128, 2, 20] f32
    #  wT4:   [128, G] bf16  (W_aug^T replicated at 32-row offsets, g rows x2)
    #  hs{m}: [128, T, 64] bf16 out — raw dump of sub-slots 1..T (h at cols
    #         32b+{12:32}; host slices).
    x_d = [
        nc.dram_tensor(
            f"x{m}", [128, T_ + 1, NBLK * KAUG], bf16, kind="ExternalInput"
        )
        for m in range(NHALF)
    ]
    cx_d = [
        nc.dram_tensor(f"cx{m}", [128, NBLK, H], f32, kind="ExternalInput")
        for m in range(NHALF)
    ]
    w_d = nc.dram_tensor("wT4", [128, G], bf16, kind="ExternalInput")
    hs_d = [
        nc.dram_tensor(
            f"hs{m}", [128, T_, NBLK * KAUG], bf16, kind="ExternalOutput"
        )
        for m in range(NHALF)
    ]

    copy_engine = "gpsimd" if "cpgps" in variant else "vector"

    with tile.TileContext(nc) as tc:
        with (
            tc.tile_pool(name="const", bufs=1) as const,
            tc.tile_pool(name="slotA", bufs=2) as slotpA,
            tc.tile_pool(name="slotB", bufs=2) as slotpB,
            tc.tile_pool(name="sgA", bufs=3) as sgpA,
            tc.tile_pool(name="sgB", bufs=3) as sgpB,
            tc.tile_pool(name="wpA", bufs=3) as wpA,
            tc.tile_pool(name="wpB", bufs=3) as wpB,
            tc.tile_pool(name="thA", bufs=3) as thpA,
            tc.tile_pool(name="thB", bufs=3) as thpB,
            tc.tile_pool(name="psgA", bufs=1, space="PSUM") as psgA,
            tc.tile_pool(name="psgB", bufs=1, space="PSUM") as psgB,
            tc.tile_pool(name="pstA", bufs=1, space="PSUM") as pstA,
            tc.tile_pool(name="pstB", bufs=1, space="PSUM") as pstB,
        ):
            slotp = [slotpA, slotpB]
            sgp = [sgpA, sgpB]
            wp = [wpA, wpB]
            thp = [thpA, thpB]
            psg = [psgA, psgB]
            pst = [pstA, pstB]

            # ---- constants ----
            ident = const.tile([128, 128], bf16)
            make_identity(nc, ident)
            wb = const.tile([128, G], bf16)
            nc.sync.dma_start(out=wb, in_=w_d[:, :])

            # persistent cell state per half [128, 2, 20] f32
            C = []
            for m in range(NHALF):
                cm = const.tile([128, NBLK, H], f32, name=f"C{m}")
                nc.sync.dma_start(out=cm, in_=cx_d[m][:, :, :])
                C.append(cm)

            # giant hb per half: [128, T+1, 64] bf16.
            # sub-slot t cols (b,q): q 0:12 = x_aug(t), q 12:32 = h(t-1).
            HB = []
            for m in range(NHALF):
                hbm = const.tile([128, T_ + 1, NBLK * KAUG], bf16, name=f"HB{m}")
                HB.append(hbm)

            def hb_slot(m, t):
                return HB[m][:, t, :]  # [128, 64]



            def hb_h(m, t):
                # h-cols of sub-slot t: [128, 2, 20]
                return HB[m].rearrange("p t (b q) -> p t b q", b=NBLK)[
                    :, t, :, IA:KAUG
                ]

            # ---- prologue: load the full hb image (x + hx + zeros) ----
            # chunked along T: a single DMA's element-count field is 16-bit
            xchunk = 128
            for m in range(NHALF):
                for t0 in range(0, T_ + 1, xchunk):
                    t1 = min(t0 + xchunk, T_ + 1)
                    nc.sync.dma_start(
                        out=HB[m][:, t0:t1, :], in_=x_d[m][:, t0:t1, :]
                    )

            # half m's transposed slot lives at partitions 64m:64m+64 so the
            # gates matmul's lhsT partition start matches its wb slice
            # (walrus: "Fmap and Weight must start at the same partition").
            def make_slot(m, t):
                pT = pst[m].tile([128, 128], bf16, tag=f"pst{m}")
                nc.tensor.transpose(
                    pT[64 * m : 64 * m + 64, :],
                    hb_slot(m, t),
                    ident,
                    tile_position=(0, 64 * m),
                )
                sl = slotp[m].tile([128, 128], bf16, tag=f"slot{m}")
                getattr(nc, copy_engine).tensor_copy(
                    sl[64 * m : 64 * m + 64, :], pT[64 * m : 64 * m + 64, :]
                )
                return sl

            slot = [make_slot(m, 0) for m in range(NHALF)]

            # ---- main loop, fully unrolled, halves interleaved ----
            # emission order is per-engine queue order: put both halves' gate
            # matmuls before either half's transpose so neither blocks the
            # other behind a not-yet-ready transpose (in-order PE queue).
            for t in range(T_):
                Ss = []
                for m in range(NHALF):
                    # gates matmuls: 2 row-grouped blocks -> 2 psum banks
                    pg = psg[m].tile([128, NBLK, 512], f32, tag=f"psg{m}")
                    for b in range(NBLK):
                        r = 64 * m + 32 * b
                        nc.tensor.matmul(
                            pg[:, b, 0:G],
                            lhsT=slot[m][r : r + KAUG, :],
                            rhs=wb[r : r + KAUG, :],
                            start=True,
                            stop=True,
                            tile_position=(r, 0),
                        )
                    # sigmoid over all gates (g rows pre-scaled by 2)
                    S = sgp[m].tile([128, NBLK, G], bf16, tag=f"sg{m}")
                    nc.scalar.activation(S, pg[:, :, 0:G], AF.Sigmoid)
                    Ss.append(S)

                for m in range(NHALF):
                    S = Ss[m]
                    Si = S[:, :, 0:20]
                    Sf = S[:, :, 20:40]
                    Sg = S[:, :, 40:60]
                    So = S[:, :, 60:80]

                    # c update: c *= f ; w = (sg-0.5)*i ; c = 2w + c
                    nc.vector.tensor_mul(C[m], Sf, C[m])
                    W = wp[m].tile([128, NBLK, H], bf16, tag=f"wp{m}")
                    nc.vector.scalar_tensor_tensor(
                        W, in0=Sg, scalar=0.5, in1=Si, op0=OP.subtract, op1=OP.mult
                    )
                    nc.vector.scalar_tensor_tensor(
                        C[m], in0=W, scalar=2.0, in1=C[m], op0=OP.mult, op1=OP.add
                    )
                    TH = thp[m].tile([128, NBLK, H], bf16, tag=f"th{m}")
                    nc.scalar.activation(TH, C[m], AF.Tanh)

                    # h(t) = o * tanh(c) -> hb sub-slot t+1 h-cols (also = hs[t])
                    nc.vector.tensor_mul(hb_h(m, t + 1), So, TH)

                    if t + 1 < T_:
                        slot[m] = make_slot(m, t + 1)

            # ---- epilogue: raw dump of sub-slots 1..T, chunked ----
            for m in range(NHALF):
                for t0 in range(0, T_, xchunk):
                    t1 = min(t0 + xchunk, T_)
                    nc.sync.dma_start(
                        out=hs_d[m][:, t0:t1, :],
                        in_=HB[m][:, t0 + 1 : t1 + 1, :],
                    )

    _split_waits(nc)
    return nc


def _get_compiled():
    global _compiled
    if _compiled is None:
        _compiled = _build_bass()
    return _compiled


def _prep_w(W_ih, W_hh, b_ih, b_hh):
    import ml_dtypes

    # augmented weight [G, KAUG]: cols 0:10 = W_ih, col 10 = bias,
    # col 11 = zero pad, cols 12:32 = W_hh
    Waug = np.zeros((G, KAUG), dtype=np.float32)
    Waug[:, 0:I] = W_ih
    Waug[:, I] = b_ih + b_hh
    Waug[:, IA:] = W_hh
    Waug[40:60, :] *= 2.0  # g rows: sigmoid(2g) trick
    wT4 = np.zeros((128, G), dtype=np.float32)
    for a in range(4):
        wT4[32 * a : 32 * a + KAUG, :] = Waug.T
    return wT4.astype(ml_dtypes.bfloat16)


def build_in_maps(x, hx, cx, W_ih, W_hh, b_ih, b_hh):
    """Host-side packing: per-core, per-half tensors as the kernel expects."""
    import ml_dtypes

    bf16 = ml_dtypes.bfloat16
    x = np.asarray(x, np.float32)
    hx = np.asarray(hx, np.float32)
    cx = np.asarray(cx, np.float32)
    wT4 = _prep_w(
        np.asarray(W_ih, np.float32),
        np.asarray(W_hh, np.float32),
        np.asarray(b_ih, np.float32),
        np.asarray(b_hh, np.float32),
    )

    # Full hb image: [core, half, 128(p), T+1, 2(b), 32]
    # sub-slot t cols 32b+{0:12} = x_aug(t) (t<T); sub-slot 0 cols
    # 32b+{12:32} = hx; else 0.  batch = core*512 + half*256 + block*128 + p
    img = np.zeros((NCORES, NHALF, 128, T + 1, NBLK, KAUG), dtype=np.float32)
    # x: [T, 4096, 10] -> [core, half, p, t, b, i]
    x6 = x.reshape(T, NCORES, NHALF, NBLK, 128, I).transpose(1, 2, 4, 0, 3, 5)
    img[:, :, :, :T, :, 0:I] = x6
    img[:, :, :, :T, :, I] = 1.0
    h6 = hx.reshape(NCORES, NHALF, NBLK, 128, H).transpose(0, 1, 3, 2, 4)
    img[:, :, :, 0, :, IA:KAUG] = h6
    img = np.ascontiguousarray(img).astype(bf16)

    c6 = cx.reshape(NCORES, NHALF, NBLK, 128, H).transpose(0, 1, 3, 2, 4)
    c6 = np.ascontiguousarray(c6).astype(np.float32)

    in_maps = []
    for k in range(NCORES):
        im = {"wT4": wT4}
        for m in range(NHALF):
            im[f"x{m}"] = img[k, m].reshape(128, T + 1, NBLK * KAUG)
            im[f"cx{m}"] = c6[k, m]
        in_maps.append(im)
    return in_maps


def unshard_output(results):
    """results: list per core of {hs0, hs1: [128,T,64] bf16} -> [T,4096,20] f32."""
    outs = np.stack(
        [
            np.stack([np.asarray(results[k][f"hs{m}"]) for m in range(NHALF)])
            for k in range(NCORES)
        ]
    ).reshape(NCORES, NHALF, 128, T, NBLK, KAUG)[:, :, :, :, :, IA:KAUG]
    # [core, half, 128(p), T, 2(b), 20]
    outs = outs.astype(np.float32).transpose(3, 0, 1, 4, 2, 5)
    # -> [T, core, half, 2(b), 128(p), 20]
    return np.ascontiguousarray(outs.reshape(T, BFULL, H))


def kernel(x, hx, cx, W_ih, W_hh, b_ih, b_hh):
    from concourse.bass_utils import run_bass_kernel_spmd

    nc = _get_compiled()
    in_maps = build_in_maps(x, hx, cx, W_ih, W_hh, b_ih, b_hh)
    res = run_bass_kernel_spmd(nc, in_maps, list(range(NCORES)))
    return unshard_output(res.results)
